# revision 14
# baseline (speedup 1.0000x reference)
# Trainium2 Bass kernel for nn_Normalization_60095182406123.
#
# Math: out = cmix(blurHW(x^2)) where
#   blurHW = separable 32-tap Gaussian over H and W (pad T16/B15/L16/R15, VALID)
#   cmix   = separable 3-tap Gaussian over (freq, orient) channel grid, zero-padded
# Input  x  [4, 192, 224, 224] f32, feat = freq*16 + orient*2 + phase
# Output    [4, 12, 8, 2, 224, 224] f32
#
# Sharding: 8 cores over (image n, phase p): each core owns x[n, p::2] =
# [96, 224, 224] — convs never cross (n, p), so no halos, no collectives.
#
# Host prep: x^2 in fp16, transposed to [h-half, 112, c, w] so every input
# DMA is a contiguous full-rate stream.  Output leaves the core as fp16 and
# is upcast on the host (rel-err budget is 2e-2; fp16 path measures ~1e-3).
#
# Per-core pipeline (3 matmul stages, all fp16 operands, f32 PSUM):
#   MM1 (H-conv): data-stationary lhsT=Xsq[112 h-half, 128 w-chunk],
#        rhs=ThA/ThB[112,128] banded pair -> PSUM [w-chunk, (i,c)]
#   MM2 (W-conv): data-stationary lhsT=Z[128 w-half, 128 c-pad] (contiguous
#        cols -> fast weight load), rhs=Tlo/Thi[128,112] disjoint half-bands
#        -> PSUM [c, (i,w')]
#   MM3 (c-mix):  const-stationary lhsT=M96[96,128], rhs=SW[96,512]
#        -> PSUM [c', (i,w')] -> OUT fp16 -> DMA
# Phase 2 runs in 7 interleaved 32-row quarters so TensorE never idles
# (HAM stays at full clock) while PSUM evacuation pipelines behind it.
import os
import sys

for _p in ("/opt/trn_rl_repo", "/root/.axon_site/_ro/trn_rl_repo"):
    if os.path.isdir(_p) and _p not in sys.path:
        sys.path.insert(0, _p)

import numpy as np

import concourse.bacc as bacc
import concourse.mybir as mybir
import concourse.tile as tile
from concourse.bass_utils import run_bass_kernel_spmd

SZ = 224          # spatial size (and conv output size)
C = 96            # channels per core (12 freq x 8 orient, fixed phase)
CP = 128          # channel dim padded (contiguous weight cols for MM2)
HC = 112          # h half (K chunk for MM1)
CG = 8            # channels per load group
NCG = C // CG     # 12
IQ = 32           # output rows per phase-2 quarter
NQ = SZ // IQ     # 7

F32 = mybir.dt.float32
F16 = mybir.dt.float16
F8 = mybir.dt.float8e4

import ml_dtypes

F8_NP = ml_dtypes.float8_e4m3fn

LAST_EXEC_NS = None


def _gauss(l):
    t = np.linspace(-1.0, 1.0, l)
    return (np.exp(-t * t / 2.0) / np.sqrt(2.0 * np.pi)).astype(np.float32)


def _make_consts():
    g32 = _gauss(32)  # H and W taps (identical)
    gsm = _gauss(3)   # freq/orient taps
    # MM1 (H-conv): x rows on partitions. out[i] = sum_a g[a] x[i + a - 16].
    # ThA: rows k = x rows 0..111, band i in [0, 128)
    # ThB: rows k = x rows 112..223, band i in [96, 224) (col j = i - 96)
    ThA = np.zeros((HC, 128), np.float32)
    ThB = np.zeros((HC, 128), np.float32)
    for k in range(HC):
        for j in range(128):
            a = k - j + 16
            if 0 <= a < 32:
                ThA[k, j] = g32[a]
            b = k + 32 - j  # (k+112) - (96+j) + 16
            if 0 <= b < 32:
                ThB[k, j] = g32[b]
    # MM2 (W-conv): disjoint half-bands from full 128-row w-windows.
    # Tlo: rows k = w 0..127,   band w' = j in [0, 112)
    # Thi: rows k = w 96..223,  band w' = 112 + j
    Tlo = np.zeros((128, HC), np.float32)
    Thi = np.zeros((128, HC), np.float32)
    for k in range(128):
        for j in range(HC):
            a = k - j + 16
            if 0 <= a < 32:
                Tlo[k, j] = g32[a]
            b = k - j  # (96+k) - (112+j) + 16
            if 0 <= b < 32:
                Thi[k, j] = g32[b]
    # channel mix (M padded to 128 cols so the weight load is full-width):
    # out[(f',o')] = sum gf[f-f'+1] go[o-o'+1] S[(f,o)]
    M96 = np.zeros((C, CP), np.float32)
    for f in range(12):
        for o in range(8):
            for fp in range(12):
                for op in range(8):
                    df, do = f - fp, o - op
                    if -1 <= df <= 1 and -1 <= do <= 1:
                        M96[f * 8 + o, fp * 8 + op] = gsm[df + 1] * gsm[do + 1]
    return (ThA.astype(np.float16), ThB.astype(np.float16),
            Tlo.astype(np.float16), Thi.astype(np.float16),
            M96.astype(np.float16))


_BUILT = None


def _build():
    global _BUILT
    if _BUILT is not None:
        return _BUILT
    ThA_np, ThB_np, Tlo_np, Thi_np, M96_np = _make_consts()

    nc = bacc.Bacc("TRN2", target_bir_lowering=False, debug=False)
    # host-prepped input: x^2 fp8-e4m3, [h-half, 112, c, w]
    xs = nc.dram_tensor("xs", [2, HC, C, SZ], F8, kind="ExternalInput")
    ys = nc.dram_tensor("ys", [C, SZ, SZ], F16, kind="ExternalOutput")
    thA_d = nc.inline_tensor(ThA_np, "ThA")
    thB_d = nc.inline_tensor(ThB_np, "ThB")
    tlo_d = nc.inline_tensor(Tlo_np, "Tlo")
    thi_d = nc.inline_tensor(Thi_np, "Thi")
    m96_d = nc.inline_tensor(M96_np, "M96")

    with tile.TileContext(nc) as tc:
        with tc.tile_pool(name="consts", bufs=1) as cp, \
             tc.tile_pool(name="zbuf", bufs=1) as zp:
            thA = cp.tile([HC, 128], F16, tag="thA")
            thB = cp.tile([HC, 128], F16, tag="thB")
            tlo = cp.tile([128, HC], F16, tag="tlo")
            thi = cp.tile([128, HC], F16, tag="thi")
            m96 = cp.tile([C, CP], F16, tag="m96")
            nc.sync.dma_start(thA[:], thA_d[:])
            nc.sync.dma_start(thB[:], thB_d[:])
            nc.sync.dma_start(tlo[:], tlo_d[:])
            nc.sync.dma_start(thi[:], thi_d[:])
            nc.sync.dma_start(m96[:], m96_d[:])

            # persistent intermediate: Z[ch] [128 w, (i 224, c 128-pad)] fp16
            # pad cols 96..127 are never written; the garbage they inject
            # lands in PSUM partitions 96..127 which are never evacuated.
            Z0 = zp.tile([128, SZ * CP], F16, tag="z0")
            Z1 = zp.tile([128, SZ * CP], F16, tag="z1")
            Zv = [Z0[:].rearrange("p (i c) -> p i c", c=CP),
                  Z1[:].rearrange("p (i c) -> p i c", c=CP)]

            # ---------------- Phase 1: load, H-conv ----------------
            with tc.tile_pool(name="xin", bufs=6) as xp, \
                 tc.tile_pool(name="ps1", bufs=4, space="PSUM") as ps1:
                for cg in range(NCG):
                    XA = xp.tile([HC, CG * SZ], F8, tag="xa")
                    XB = xp.tile([HC, CG * SZ], F8, tag="xb")
                    # first group: split the load so MM1 starts sooner
                    nsp = 4 if cg == 0 else 1
                    for sp in range(nsp):
                        cw = CG // nsp
                        c0 = cg * CG + sp * cw
                        nc.sync.dma_start(
                            XA[:, sp * cw * SZ:(sp + 1) * cw * SZ].rearrange(
                                "p (c w) -> p c w", c=cw),
                            xs[0, :, c0:c0 + cw, :])
                        nc.sync.dma_start(
                            XB[:, sp * cw * SZ:(sp + 1) * cw * SZ].rearrange(
                                "p (c w) -> p c w", c=cw),
                            xs[1, :, c0:c0 + cw, :])
                    for ch in range(2):  # w-chunk: 0..127 / 96..223
                        for cq in range(2):  # 4 channels per psum tile
                            P1 = ps1.tile([128, 1024], F32, tag="p1")
                            for cl4 in range(4):
                                cl = cq * 4 + cl4
                                col = cl * SZ + ch * C
                                off = (cl4 // 2) * 512 + (cl4 % 2) * SZ
                                nc.tensor.matmul(
                                    P1[:, off:off + 128],
                                    XA[:, col:col + 128], thA[:],
                                    start=True, stop=False)
                                nc.tensor.matmul(
                                    P1[:, off + 96:off + 224],
                                    XB[:, col:col + 128], thB[:],
                                    start=False, stop=True)
                            # iterate (i, c): dst writes become 16B-aligned
                            # contiguous runs; the stride lands on the PSUM
                            # read side instead of the SBUF write side
                            src_ap = P1[:].rearrange(
                                "p (q x) -> p q x", x=512)[:, :, 0:448].rearrange(
                                "p q (c i) -> p i q c", i=SZ)
                            dst_ap = Zv[ch][:, :,
                                            cg * CG + cq * 4:cg * CG + cq * 4 + 4]
                            if (cg + ch + cq) % 2 == 0:
                                nc.vector.tensor_copy(dst_ap, src_ap)
                            else:
                                nc.scalar.copy(dst_ap, src_ap)

            # ------------- Phase 2: W-conv, channel mix, store -------------
            # 7 quarters of 32 rows; MM2 and MM3 interleave so TensorE
            # always has work while DVE/ACT drain PSUM behind it.
            with tc.tile_pool(name="sw", bufs=2) as swp, \
                 tc.tile_pool(name="outp", bufs=2) as outp, \
                 tc.tile_pool(name="ps2", bufs=2, space="PSUM") as ps2, \
                 tc.tile_pool(name="ps3", bufs=2, space="PSUM") as ps3:
                for q in range(NQ):
                    i0 = q * IQ
                    SW = swp.tile([C, IQ * SZ], F16, tag="sw")
                    SWv = SW[:].rearrange("p (i w) -> p i w", w=SZ)
                    for it in range(IQ // 4):  # 8 psum tiles of 4 rows
                        P2 = ps2.tile([128, 1024], F32, tag="p2")
                        for il in range(4):
                            i = i0 + it * 4 + il
                            off = (il // 2) * 512 + (il % 2) * SZ
                            nc.tensor.matmul(
                                P2[:, off:off + 112],
                                Zv[0][:, i, :], tlo[:],
                                start=True, stop=True)
                            nc.tensor.matmul(
                                P2[:, off + 112:off + 224],
                                Zv[1][:, i, :], thi[:],
                                start=True, stop=True)
                        src_ap = P2[0:C].rearrange(
                            "p (q2 x) -> p q2 x", x=512)[:, :, 0:448]
                        dst_ap = SWv[:, it * 4:(it + 1) * 4, :]
                        if it % 2 == 0:
                            nc.scalar.copy(dst_ap, src_ap)
                        else:
                            nc.vector.tensor_copy(dst_ap, src_ap)
                    OUT = outp.tile([C, IQ * SZ], F16, tag="out")
                    # IQ*SZ = 7168 = 14 * 512 -> 7 psum pair-tiles
                    for nt in range(7):
                        P3 = ps3.tile([128, 1024], F32, tag="p3")
                        base = nt * 1024
                        for h in range(2):
                            nc.tensor.matmul(
                                P3[:, h * 512:(h + 1) * 512],
                                m96[:], SW[:, base + h * 512:base + (h + 1) * 512],
                                start=True, stop=True)
                        if nt % 2 == 0:
                            nc.scalar.copy(
                                OUT[:, base:base + 1024], P3[0:C, :])
                        else:
                            nc.vector.tensor_copy(
                                OUT[:, base:base + 1024], P3[0:C, :])
                        # stream whole finished rows out as copies complete
                        if nt in (1, 3, 5):
                            r0 = [0, 9, 18][nt // 2]
                            r1 = (nt + 1) * 1024 // SZ
                            nc.sync.dma_start(
                                ys[:, i0 + r0:i0 + r1, :].rearrange(
                                    "c i w -> c (i w)"),
                                OUT[:, r0 * SZ:r1 * SZ])
                    nc.sync.dma_start(
                        ys[:, i0 + 27:i0 + IQ, :].rearrange("c i w -> c (i w)"),
                        OUT[:, 27 * SZ:IQ * SZ])

    nc.compile()
    _BUILT = nc
    return nc


def _prep_core(x_core: np.ndarray) -> np.ndarray:
    # x_core [96, 224, 224] f32 -> x^2 fp8 [2, 112, 96, 224] (h-half major)
    xsq = (x_core * x_core).astype(F8_NP)  # [c, h, w]
    xt = np.ascontiguousarray(xsq.transpose(1, 0, 2))  # [h, c, w]
    return xt.reshape(2, HC, C, SZ)


def kernel(x: np.ndarray) -> np.ndarray:
    assert x.shape == (4, 192, 224, 224) and x.dtype == np.float32
    nc = _build()
    in_maps = []
    for core in range(8):
        n, p = core // 2, core % 2
        in_maps.append({"xs": _prep_core(x[n, p::2])})
    res = run_bass_kernel_spmd(nc, in_maps, core_ids=list(range(8)))
    global LAST_EXEC_NS
    LAST_EXEC_NS = res.exec_time_ns
    out = np.empty((4, 12, 8, 2, 224, 224), np.float32)
    for core in range(8):
        n, p = core // 2, core % 2
        out[n, :, :, p] = res.results[core]["ys"].astype(np.float32).reshape(
            12, 8, 224, 224)
    return out


# revision 16
# speedup vs baseline: 1.1185x; 1.1185x over previous
# Trainium2 Bass kernel for nn_Normalization_60095182406123 — v6.
#
# Math: out = cmix(blurHW(x^2));  see v5 header.  Differences vs v5:
#  * MM1 uses K=128 h-windows {0..127, 96..223} with DISJOINT output bands
#    (same Tlo/Thi Toeplitz pair as MM2 — the taps are identical), so the
#    H-conv for rows 0..111 completes before window 1 is even loaded.
#  * Window-1 H-conv is interleaved with quarters 0..2 of phase 2 (their
#    Z rows are already complete), so TensorE and the evacuation engines
#    stay busy through what used to be a serial phase boundary.
# Input  x [4,192,224,224] f32 -> host: x^2 fp8e4m3, [2 win, 128, 96, 224]
# Output [4,12,8,2,224,224] f32 (device writes fp16, host upcasts).
import os
import sys

for _p in ("/opt/trn_rl_repo", "/root/.axon_site/_ro/trn_rl_repo"):
    if os.path.isdir(_p) and _p not in sys.path:
        sys.path.insert(0, _p)

import numpy as np

import concourse.bacc as bacc
import concourse.mybir as mybir
import concourse.tile as tile
from concourse.bass_utils import run_bass_kernel_spmd

SZ = 224          # spatial size (and conv output size)
C = 96            # channels per core (12 freq x 8 orient, fixed phase)
CP = 128          # channel dim padded (contiguous weight cols for MM2)
HB = 112          # half band (output rows per h-window / w'-half)
CG = 8            # channels per load group
NCG = C // CG     # 12
IQ = 32           # output rows per phase-2 quarter
NQ = SZ // IQ     # 7

F32 = mybir.dt.float32
F16 = mybir.dt.float16
F8 = mybir.dt.float8e4

import ml_dtypes

F8_NP = ml_dtypes.float8_e4m3fn

LAST_EXEC_NS = None


def _gauss(l):
    t = np.linspace(-1.0, 1.0, l)
    return (np.exp(-t * t / 2.0) / np.sqrt(2.0 * np.pi)).astype(np.float32)


def _make_consts():
    g32 = _gauss(32)  # H and W taps (identical)
    gsm = _gauss(3)   # freq/orient taps
    # Banded Toeplitz for a 32-tap conv with pad 16/15 from a 128-row
    # window of the source axis.  Tlo: rows = src 0..127, band out 0..111.
    # Thi: rows = src 96..223, band out 112..223 (col j = out - 112).
    Tlo = np.zeros((128, HB), np.float32)
    Thi = np.zeros((128, HB), np.float32)
    for k in range(128):
        for j in range(HB):
            a = k - j + 16
            if 0 <= a < 32:
                Tlo[k, j] = g32[a]
            b = k - j  # (96+k) - (112+j) + 16
            if 0 <= b < 32:
                Thi[k, j] = g32[b]
    # channel mix (M padded to 128 cols for a full-width weight load)
    M96 = np.zeros((C, CP), np.float32)
    for f in range(12):
        for o in range(8):
            for fp in range(12):
                for op in range(8):
                    df, do = f - fp, o - op
                    if -1 <= df <= 1 and -1 <= do <= 1:
                        M96[f * 8 + o, fp * 8 + op] = gsm[df + 1] * gsm[do + 1]
    return Tlo.astype(np.float16), Thi.astype(np.float16), M96.astype(np.float16)


_BUILT = None


def _build():
    global _BUILT
    if _BUILT is not None:
        return _BUILT
    Tlo_np, Thi_np, M96_np = _make_consts()

    nc = bacc.Bacc("TRN2", target_bir_lowering=False, debug=False)
    # host-prepped input: x^2 fp8-e4m3, [win, 128 h, c, w]
    xs = nc.dram_tensor("xs", [2, 128, C, SZ], F8, kind="ExternalInput")
    ys = nc.dram_tensor("ys", [C, SZ, SZ], F16, kind="ExternalOutput")
    tlo_d = nc.inline_tensor(Tlo_np, "Tlo")
    thi_d = nc.inline_tensor(Thi_np, "Thi")
    m96_d = nc.inline_tensor(M96_np, "M96")

    with tile.TileContext(nc) as tc:
        with tc.tile_pool(name="consts", bufs=1) as cp, \
             tc.tile_pool(name="zbuf", bufs=1) as zp, \
             tc.tile_pool(name="xin", bufs=6) as xp, \
             tc.tile_pool(name="sw", bufs=4) as swp, \
             tc.tile_pool(name="outp", bufs=2) as outp:
            tlo = cp.tile([128, HB], F16, tag="tlo")
            thi = cp.tile([128, HB], F16, tag="thi")
            m96 = cp.tile([C, CP], F16, tag="m96")
            nc.sync.dma_start(tlo[:], tlo_d[:])
            nc.sync.dma_start(thi[:], thi_d[:])
            nc.sync.dma_start(m96[:], m96_d[:])

            # persistent intermediate: Z[ch] [128 w, (i 224, c 128-pad)] fp16
            Z0 = zp.tile([128, SZ * CP], F16, tag="z0")
            Z1 = zp.tile([128, SZ * CP], F16, tag="z1")
            Zv = [Z0[:].rearrange("p (i c) -> p i c", c=CP),
                  Z1[:].rearrange("p (i c) -> p i c", c=CP)]

            ecnt = [0]

            def evac(dst_ap, src_ap):
                # alternate engines; scalar is a bit faster per element
                if ecnt[0] % 2 == 0:
                    nc.scalar.copy(dst_ap, src_ap)
                else:
                    nc.vector.tensor_copy(dst_ap, src_ap)
                ecnt[0] += 1

            def mm1_group(ps1, win, cg, rhs, ih):
                # load one channel group of one h-window, H-conv its 8
                # channels into Z rows [ih*112, (ih+1)*112)
                XA = xp.tile([128, CG * SZ], F8, tag="xa")
                nsp = 4 if (win == 0 and cg == 0) else 1
                for sp in range(nsp):
                    cw = CG // nsp
                    c0 = cg * CG + sp * cw
                    nc.sync.dma_start(
                        XA[:, sp * cw * SZ:(sp + 1) * cw * SZ].rearrange(
                            "p (c w) -> p c w", c=cw),
                        xs[win, :, c0:c0 + cw, :])
                for ch in range(2):  # w-chunk 0..127 / 96..223
                    P1 = ps1.tile([128, 1024], F32, tag="p1")
                    for cl in range(CG):
                        col = cl * SZ + ch * C
                        off = (cl // 4) * 512 + (cl % 4) * HB
                        nc.tensor.matmul(
                            P1[:, off:off + HB],
                            XA[:, col:col + 128], rhs[:],
                            start=True, stop=True)
                    src_ap = P1[:].rearrange(
                        "p (q x) -> p q x", x=512)[:, :, 0:448].rearrange(
                        "p q (c i) -> p i q c", i=HB)
                    dst_ap = Zv[ch][:, ih * HB:(ih + 1) * HB,
                                    cg * CG:(cg + 1) * CG]
                    evac(dst_ap, src_ap)

            def mm2_tile(ps2, SWv, i0, it):
                # W-conv 4 output rows into SW
                P2 = ps2.tile([128, 1024], F32, tag="p2")
                for il in range(4):
                    i = i0 + it * 4 + il
                    off = (il // 2) * 512 + (il % 2) * SZ
                    nc.tensor.matmul(
                        P2[:, off:off + 112],
                        Zv[0][:, i, :], tlo[:],
                        start=True, stop=True)
                    nc.tensor.matmul(
                        P2[:, off + 112:off + 224],
                        Zv[1][:, i, :], thi[:],
                        start=True, stop=True)
                src_ap = P2[0:C].rearrange(
                    "p (q x) -> p q x", x=512)[:, :, 0:448]
                evac(SWv[:, it * 4:(it + 1) * 4, :], src_ap)

            def mm3_tile(ps3, SW, OUT, q, nt):
                # channel mix for 1024 output elements + stream rows out
                P3 = ps3.tile([128, 1024], F32, tag="p3")
                base = nt * 1024
                for h in range(2):
                    nc.tensor.matmul(
                        P3[:, h * 512:(h + 1) * 512],
                        m96[:], SW[:, base + h * 512:base + (h + 1) * 512],
                        start=True, stop=True)
                evac(OUT[:, base:base + 1024], P3[0:C, :])
                i0 = q * IQ
                if nt in (1, 3, 5):
                    r0 = [0, 9, 18][nt // 2]
                    r1 = (nt + 1) * 1024 // SZ
                    nc.sync.dma_start(
                        ys[:, i0 + r0:i0 + r1, :].rearrange("c i w -> c (i w)"),
                        OUT[:, r0 * SZ:r1 * SZ])
                elif nt == 6:
                    nc.sync.dma_start(
                        ys[:, i0 + 27:i0 + IQ, :].rearrange("c i w -> c (i w)"),
                        OUT[:, 27 * SZ:IQ * SZ])

            # ---- phase A: h-window 0 -> Z rows 0..111 ----
            with tc.tile_pool(name="psA", bufs=4, space="PSUM") as psA:
                for cg in range(NCG):
                    mm1_group(psA, 0, cg, tlo, 0)

            # ---- phase B: h-window 1 interleaved with W-conv of
            #      quarters 0..2 (Z rows 0..95 are complete) ----
            SWs = [None] * NQ
            SWvs = [None] * NQ
            for q in range(3):
                SW_t = swp.tile([C, IQ * SZ], F16, tag="sw")
                SWs[q] = SW_t
                SWvs[q] = SW_t[:].rearrange("p (i w) -> p i w", w=SZ)
            with tc.tile_pool(name="ps1", bufs=2, space="PSUM") as ps1, \
                 tc.tile_pool(name="ps2", bufs=2, space="PSUM") as ps2:
                for cg in range(NCG):
                    mm1_group(ps1, 1, cg, thi, 1)
                    for t in range(2):
                        it = cg * 2 + t
                        mm2_tile(ps2, SWvs[it // 8], (it // 8) * IQ, it % 8)

            # ---- phase C: mix quarters 0..2, full quarters 3..6 ----
            with tc.tile_pool(name="ps2", bufs=2, space="PSUM") as ps2, \
                 tc.tile_pool(name="ps3", bufs=2, space="PSUM") as ps3:
                # mix of the three ready quarters first (PE warm-dense)
                for q in range(3):
                    OUT = outp.tile([C, IQ * SZ], F16, tag="out")
                    for nt in range(7):
                        mm3_tile(ps3, SWs[q], OUT, q, nt)
                for q in range(3, NQ):
                    SW = swp.tile([C, IQ * SZ], F16, tag="sw")
                    SWv = SW[:].rearrange("p (i w) -> p i w", w=SZ)
                    for it in range(IQ // 4):
                        mm2_tile(ps2, SWv, q * IQ, it)
                    OUT = outp.tile([C, IQ * SZ], F16, tag="out")
                    for nt in range(7):
                        mm3_tile(ps3, SW, OUT, q, nt)

    nc.compile()
    _BUILT = nc
    return nc


def _prep_core(x_core: np.ndarray) -> np.ndarray:
    # x_core [96, 224, 224] f32 -> x^2 fp8 [2, 128, 96, 224] h-windows
    xsq = (x_core * x_core).astype(F8_NP)  # [c, h, w]
    xt = np.ascontiguousarray(xsq.transpose(1, 0, 2))  # [h, c, w]
    out = np.empty((2, 128, C, SZ), F8_NP)
    out[0] = xt[0:128]
    out[1] = xt[96:224]
    return out


def kernel(x: np.ndarray) -> np.ndarray:
    assert x.shape == (4, 192, 224, 224) and x.dtype == np.float32
    nc = _build()
    in_maps = []
    for core in range(8):
        n, p = core // 2, core % 2
        in_maps.append({"xs": _prep_core(x[n, p::2])})
    res = run_bass_kernel_spmd(nc, in_maps, core_ids=list(range(8)))
    global LAST_EXEC_NS
    LAST_EXEC_NS = res.exec_time_ns
    out = np.empty((4, 12, 8, 2, 224, 224), np.float32)
    for core in range(8):
        n, p = core // 2, core % 2
        out[n, :, :, p] = res.results[core]["ys"].astype(np.float32).reshape(
            12, 8, 224, 224)
    return out
